# revision 20
# baseline (speedup 1.0000x reference)
"""Trainium2 kernel for nn_Dense_RBS_density_3D.

The reference applies 39 RBS gates sequentially to a batch of 64 density
matrices: rho <- U_g rho U_g^T. The gates compose, so the whole circuit is a
single orthogonal matrix V = U_38 @ ... @ U_0 (depends only on the 39 scalar
angles + the fixed sparsity structure), and the output is V @ rho @ V^T per
batch element.

Host side: build V from the angles (39 sparse pair-rotation sweeps applied to
an identity matrix). V inherits strong structural sparsity with geometric
magnitude decay from the adjacent-qubit gate ladder. The host computes, per
(contraction-tile, PSUM-bank), the column interval of V^T that carries
significant mass; everything outside is skipped on device (perturbs V by
~4e-3 in Frobenius norm — the bf16 noise floor is ~3.8e-3 and the gate 2e-2).

Device side (8 NeuronCores, data-parallel over batch): per batch element
compute Y = V X V^T as two transpose-free matmul passes of the same shape:

    f(Z) = Z^T @ V^T   (lhsT = Z with contraction on partitions, rhs = V^T)
    Y = f(f(X))        since (X^T V^T)^T V^T = V X V^T

bf16 operands (X pre-cast + pre-packed on host) with fp32 PSUM accumulation.
Pass 2 of batch b-1 is software-pipelined against pass 1 of batch b at
m-chunk granularity so PSUM-evacuation latency and the "pass 2 needs all 7
pchunks" dependency never stall the PE. X is packed on host into a
[128, 6, 780] per-batch layout so each load descriptor is 9.4 KB/partition
instead of a 1.5 KB row. The ragged last k-chunk (12 rows) runs as K=12
matmuls — no zero padding or memzero needed anywhere.
"""

import numpy as np
import ml_dtypes

D = 780           # binom(40, 2)
N_GATES = 39
B_TOTAL = 64
N_CORES = 8
B_LOC = B_TOTAL // N_CORES
P = 128
KT = (D + P - 1) // P          # 7 k-chunks: 6x128 + 12
LAST = D - (KT - 1) * P        # 12
FULL = (KT - 1) * P            # 768
CHUNKS = [(i * P, min(P, D - i * P)) for i in range(KT)]
BANKS = [(0, 512), (512, D)]   # PSUM fp32 bank col ranges
DROP_BUDGET = 4e-3             # allowed relative Frobenius perturbation of V

_CACHE = {}


def _build_V(angles, Bmat):
    """V = U_38 @ ... @ U_0 in float64, where U_g = cos(th) A + sin(th) B + C.

    B[g, j, i] == +1 identifies the coupled pair (i, j): U[i,i]=U[j,j]=cos,
    U[j,i]=+sin, U[i,j]=-sin; all other rows are identity.
    """
    V = np.eye(D, dtype=np.float64)
    for g in range(N_GATES):
        jj, ii = np.nonzero(Bmat[g] > 0.5)
        c = np.cos(float(angles[g]))
        s = np.sin(float(angles[g]))
        Vi = V[ii, :].copy()
        Vj = V[jj, :].copy()
        V[ii, :] = c * Vi - s * Vj
        V[jj, :] = s * Vi + c * Vj
    return V


def _plan_intervals(V):
    """Per (k-tile, PSUM bank): [c0, c1) column interval of V^T holding all
    significant mass, or None.
    """
    VT = V.T  # [k, n] — the rhs layout
    sliver = np.zeros((KT, D))
    for kc, (k0, ksz) in enumerate(CHUNKS):
        sliver[kc] = (VT[k0:k0 + ksz, :] ** 2).sum(axis=0)
    tot = sliver.sum()
    flat = np.sort(sliver.ravel())
    csum = np.cumsum(flat)
    budget = DROP_BUDGET ** 2 * tot
    pos = np.searchsorted(csum, budget)
    thr = flat[pos - 1] if pos > 0 else -1.0
    sig = sliver > thr

    intervals = []  # [kc][bank] -> (c0, c1) or None
    for kc in range(KT):
        row = []
        for b0, b1 in BANKS:
            cols = np.nonzero(sig[kc, b0:b1])[0]
            if len(cols) == 0:
                row.append(None)
                continue
            c0 = int(b0 + cols[0]) & ~1          # 8-byte-align start
            c1 = min(b1, (int(b0 + cols[-1]) + 2) & ~1)
            row.append((c0, c1))
        intervals.append(row)

    # safety: every column must be covered by at least one kept interval,
    # else the PSUM evacuation would read stale garbage there.
    covered = np.zeros(D, bool)
    for row in intervals:
        for iv in row:
            if iv is not None:
                covered[iv[0]:iv[1]] = True
    if not covered.all():
        for bi, (b0, b1) in enumerate(BANKS):
            if not covered[b0:b1].all():
                kc = int(sliver[:, b0:b1].sum(axis=1).argmax())
                intervals[kc][bi] = (b0, b1)
    return intervals


def _build_program(intervals):
    import concourse.bacc as bacc
    import concourse.mybir as mybir
    import concourse.tile as tile

    nc = bacc.Bacc("TRN2", target_bir_lowering=False, debug=False,
                   num_devices=N_CORES)
    bf16 = mybir.dt.bfloat16
    f32 = mybir.dt.float32

    # host-packed X: x1[b, p, kc, n] = X[b, kc*128+p, n] for kc<6 (so each
    # partition's slice is 9360 B contiguous), x2[b] = rows 768:780.
    x1 = nc.dram_tensor("x1", [B_LOC, P, KT - 1, D], bf16,
                        kind="ExternalInput").ap()
    x2 = nc.dram_tensor("x2", [B_LOC, LAST, D], bf16,
                        kind="ExternalInput").ap()
    vt1 = nc.dram_tensor("vt1", [P, KT - 1, D], bf16,
                         kind="ExternalInput").ap()
    vt2 = nc.dram_tensor("vt2", [LAST, D], bf16,
                         kind="ExternalInput").ap()
    # packed output: chunk pairs (0,1),(2,3),(4,5) -> y1[b, jc, p, t, n] holds
    # row 256*jc + 128*t + p; the 12-row tail goes to y2. Host unpacks.
    y1 = nc.dram_tensor("y1", [B_LOC, 3, P, 2, D], bf16,
                        kind="ExternalOutput").ap()
    y2 = nc.dram_tensor("y2", [B_LOC, LAST, D], bf16,
                        kind="ExternalOutput").ap()

    # flat list of kept (kc, bank_idx, c0, c1). k-chunks are ordered
    # narrow-first / wide-last (bank pairs adjacent): the wide matmuls at the
    # group tail keep the PE streaming while the weight port preloads the
    # next group's early (cheap) k-chunks — otherwise the group boundary
    # serializes on LDWEIGHTS. first/last flags follow emission order.
    width = {kc: sum(iv[1] - iv[0] for iv in intervals[kc] if iv)
             for kc in range(KT)}
    kc_order = sorted(range(KT), key=lambda kc: width[kc])
    kept = [(kc, bi, iv[0], iv[1])
            for kc in kc_order for bi, iv in enumerate(intervals[kc])
            if iv is not None]
    first_kc = {}
    last_kc = {}
    for kc, bi, _, _ in kept:
        first_kc.setdefault(bi, kc)
        last_kc[bi] = kc

    BK0 = 512                       # PSUM bank boundary in fp32 cols

    with tile.TileContext(nc) as tc:
        with (
            tc.tile_pool(name="vtp", bufs=1) as vtp,
            tc.tile_pool(name="xb", bufs=3) as xbp,
            tc.tile_pool(name="pt", bufs=2) as ptp,
            tc.tile_pool(name="yo", bufs=3) as yop,
            tc.tile_pool(name="wup", bufs=1) as wup,
            tc.tile_pool(name="ps", bufs=4, space="PSUM") as psp,
        ):
            # PE warmup: a few dummy matmuls start the HAM clock ramp while
            # the first DMAs land; they use psum-pool generations that
            # rotate away before real work needs them.
            wz = wup.tile([P, 256], bf16)
            nc.vector.memset(wz[:], 0.0)
            ps_w = psp.tile([P, D], f32, tag="ps")
            for _ in range(6):
                nc.tensor.matmul(ps_w[:, :256], wz[:, :P], wz[:, :256],
                                 start=True, stop=True)

            # V^T resident in SBUF, k-partitioned: vt_sb[p, kc, n]. The pad
            # partitions [LAST:, KT-1, :] are never read (kc=6 matmuls use
            # K=12), so no memzero is needed.
            vt_sb = vtp.tile([P, KT, D], bf16)

            grp_idx = 0

            def evac(out_ap, psum_ap, parity):
                # rotate PSUM evacuation between ScalarE and VectorE so each
                # engine alternates the wide (512-col) and narrow (268-col)
                # bank copies across groups
                if parity % 2 == 0:
                    nc.scalar.copy(out_ap, psum_ap)
                else:
                    nc.vector.tensor_copy(out=out_ap, in_=psum_ap)

            def pass_mms(ps, src_fn, msz):
                for kc, bi, c0, c1 in kept:
                    ksz = P if kc < KT - 1 else LAST
                    nc.tensor.matmul(
                        ps[:msz, c0:c1],
                        src_fn(kc)[:ksz],
                        vt_sb[:ksz, kc, c0:c1],
                        start=(kc == first_kc[bi]),
                        stop=(kc == last_kc[bi]),
                    )

            def load_x(b):
                xb = xbp.tile([P, KT, D], bf16, tag="xb")
                nc.sync.dma_start(xb[:, : KT - 1, :], x1[b])
                nc.sync.dma_start(xb[:LAST, KT - 1, :], x2[b])
                return xb

            def load_x0():
                # startup: load V^T and X0 chunk-by-chunk in matmul emission
                # order so the first group starts (subtile deps) after ~0.4 MB
                # instead of after all of V^T + X0.
                xb = xbp.tile([P, KT, D], bf16, tag="xb")
                for kc in kc_order:
                    if kc == KT - 1:
                        nc.sync.dma_start(vt_sb[:LAST, kc, :], vt2)
                        nc.sync.dma_start(xb[:LAST, kc, :], x2[0])
                    else:
                        nc.sync.dma_start(vt_sb[:, kc, :], vt1[:, kc, :])
                        nc.sync.dma_start(xb[:, kc, :], x1[0, :, kc, :])
                return xb

            EV0 = 392     # balanced evac split point (copies may span banks)

            def emit_p1(mc, xb, pch):
                nonlocal grp_idx
                m0, msz = CHUNKS[mc]
                ps = psp.tile([P, D], f32, tag="ps")
                pass_mms(ps, lambda kc: xb[:, kc, m0:m0 + msz], msz)
                evac(pch[mc][:msz, :EV0], ps[:msz, :EV0], grp_idx)
                evac(pch[mc][:msz, EV0:], ps[:msz, EV0:], grp_idx + 1)
                grp_idx += 1

            def emit_p2(b, ic, pch, yo_pair):
                nonlocal grp_idx
                i0, isz = CHUNKS[ic]
                ps = psp.tile([P, D], f32, tag="ps")
                pass_mms(ps, lambda kc: pch[kc][:, i0:i0 + isz], isz)
                if ic == KT - 1:
                    yo = yop.tile([LAST, D], bf16, tag="yot")
                    evac(yo[:isz, :EV0], ps[:isz, :EV0], grp_idx)
                    evac(yo[:isz, EV0:], ps[:isz, EV0:], grp_idx + 1)
                    grp_idx += 1
                    nc.sync.dma_start(y2[b], yo[:isz, :])
                    return None
                if yo_pair is None:
                    yo_pair = yop.tile([P, 2, D], bf16, tag="yo")
                t = ic % 2
                evac(yo_pair[:isz, t, :EV0], ps[:isz, :EV0], grp_idx)
                evac(yo_pair[:isz, t, EV0:], ps[:isz, EV0:], grp_idx + 1)
                grp_idx += 1
                if t == 1:
                    nc.sync.dma_start(y1[b, ic // 2], yo_pair[:])
                    return None
                return yo_pair

            # software pipeline: slot b runs pass-1 of batch b interleaved
            # with pass-2 of batch b-1. Pass-2 chunks trail by 2 so the last
            # pass-1 evacuations of batch b-1 have landed before pass-2
            # needs pchunk[5] and [6].
            ORDER = [("p1", 0), ("p1", 1)]
            for i in range(2, KT):
                ORDER += [("p1", i), ("p2", i - 2)]
            ORDER += [("p2", KT - 2), ("p2", KT - 1)]

            xb_cur = load_x0()
            pch_prev = None
            yo_pair = None
            for b in range(B_LOC):
                pch_cur = [ptp.tile([P, D], bf16, tag=f"pt{i}", name=f"pc{i}")
                           for i in range(KT)]
                if b + 1 < B_LOC:
                    xb_next = load_x(b + 1)
                for kind, i in ORDER:
                    if kind == "p1":
                        emit_p1(i, xb_cur, pch_cur)
                    elif pch_prev is not None:
                        yo_pair = emit_p2(b - 1, i, pch_prev, yo_pair)
                pch_prev = pch_cur
                xb_cur = xb_next
            for i in range(KT):
                yo_pair = emit_p2(B_LOC - 1, i, pch_prev, yo_pair)

    nc.compile()
    _dedupe_ldweights(nc)
    return nc


def _dedupe_ldweights(nc):
    """Drop an InstLdweights whose weights AP is identical to the previous
    one with only PE matmuls in between — the weights are already resident
    in the PE array, so the reload only burns weight-port bandwidth (the
    group bottleneck). Only sync-free loads are dropped, and matmul
    semaphore updates are untouched, so the schedule's counts are
    preserved. Runs post-compile, pre-serialization.
    """
    import concourse.mybir as mybir

    removed = 0
    for blk in nc.main_func.blocks:
        insts = blk.instructions
        last_key = None
        drop = []
        for x in insts:
            if isinstance(x, mybir.InstLdweights):
                si = x.sync_info
                clean = si is None or (len(si.on_wait) == 0
                                       and len(si.on_update) == 0)
                key = str(x.ins[0])
                if clean and key == last_key:
                    drop.append(x)
                    continue
                last_key = key
            elif not isinstance(x, mybir.InstMatmult):
                # conservatively assume anything else on the PE engine (or
                # control flow) may disturb the loaded weights
                eng = getattr(x, "engine", None)
                if eng is None or "PE" in str(eng):
                    last_key = None
        for x in drop:
            insts.remove(x)
        removed += len(drop)
    return removed


def _get_program(intervals):
    key = tuple(tuple(row) for row in intervals)
    if _CACHE.get("key") != key:
        _CACHE["nc"] = _build_program(intervals)
        _CACHE["key"] = key
    return _CACHE["nc"]


def kernel(input_state, angles, A, B, C, _trace=False):
    from concourse.bass_utils import run_bass_kernel_spmd

    X = np.asarray(input_state, dtype=np.float32)
    V = _build_V(np.asarray(angles, dtype=np.float64), np.asarray(B))
    vt = np.ascontiguousarray(V.T).astype(ml_dtypes.bfloat16)
    X_bf = X.astype(ml_dtypes.bfloat16)

    # pack: x1[b, p, kc, n] = X[b, kc*128+p, n]
    x1 = np.ascontiguousarray(
        X_bf[:, :FULL].reshape(B_TOTAL, KT - 1, P, D).transpose(0, 2, 1, 3))
    x2 = np.ascontiguousarray(X_bf[:, FULL:])
    vt1 = np.ascontiguousarray(
        vt[:FULL].reshape(KT - 1, P, D).transpose(1, 0, 2))
    vt2 = np.ascontiguousarray(vt[FULL:])

    intervals = _plan_intervals(V)
    nc = _get_program(intervals)
    in_maps = [
        {"x1": x1[c * B_LOC:(c + 1) * B_LOC],
         "x2": x2[c * B_LOC:(c + 1) * B_LOC],
         "vt1": vt1, "vt2": vt2}
        for c in range(N_CORES)
    ]
    res = run_bass_kernel_spmd(nc, in_maps, core_ids=list(range(N_CORES)),
                               trace=_trace)
    out = np.empty((B_TOTAL, D, D), np.float32)
    for c in range(N_CORES):
        # y1[b, jc, p, t, n] holds row 256*jc + 128*t + p; y2 holds 768:780
        y1 = np.asarray(res.results[c]["y1"], dtype=np.float32)
        y2 = np.asarray(res.results[c]["y2"], dtype=np.float32)
        sl = out[c * B_LOC:(c + 1) * B_LOC]
        sl[:, :FULL] = y1.transpose(0, 1, 3, 2, 4).reshape(B_LOC, FULL, D)
        sl[:, FULL:] = y2
    if _trace:
        kernel.last_results = res
    return out


# revision 27
# speedup vs baseline: 1.1752x; 1.1752x over previous
"""Trainium2 kernel for nn_Dense_RBS_density_3D.

The reference applies 39 RBS gates sequentially to a batch of 64 density
matrices: rho <- U_g rho U_g^T. The gates compose, so the whole circuit is a
single orthogonal matrix V = U_38 @ ... @ U_0 (depends only on the 39 scalar
angles + the fixed sparsity structure), and the output is V @ rho @ V^T per
batch element.

Host side: build V from the angles (39 sparse pair-rotation sweeps applied to
an identity matrix). V inherits strong structural sparsity with geometric
magnitude decay from the adjacent-qubit gate ladder. The host computes, per
(contraction-tile, PSUM-bank), the column interval of V^T that carries
significant mass; everything outside is skipped on device (perturbs V by
~4e-3 in Frobenius norm — the bf16 noise floor is ~3.8e-3 and the gate 2e-2).

Device side (8 NeuronCores, data-parallel over batch): per batch element
compute Y = V X V^T as two transpose-free matmul passes of the same shape:

    f(Z) = Z^T @ V^T   (lhsT = Z with contraction on partitions, rhs = V^T)
    Y = f(f(X))        since (X^T V^T)^T V^T = V X V^T

bf16 operands (X pre-cast + pre-packed on host) with fp32 PSUM accumulation.
Pass 2 of batch b-1 is software-pipelined against pass 1 of batch b at
m-chunk granularity so PSUM-evacuation latency and the "pass 2 needs all 7
pchunks" dependency never stall the PE. X is packed on host into a
[128, 6, 780] per-batch layout so each load descriptor is 9.4 KB/partition
instead of a 1.5 KB row. The ragged last k-chunk (12 rows) runs as K=12
matmuls — no zero padding or memzero needed anywhere.
"""

import numpy as np
import ml_dtypes

D = 780           # binom(40, 2)
N_GATES = 39
B_TOTAL = 64
N_CORES = 8
B_LOC = B_TOTAL // N_CORES
P = 128
KT = (D + P - 1) // P          # 7 k-chunks: 6x128 + 12
LAST = D - (KT - 1) * P        # 12
FULL = (KT - 1) * P            # 768
CHUNKS = [(i * P, min(P, D - i * P)) for i in range(KT)]
BANKS = [(0, 512), (512, D)]   # PSUM fp32 bank col ranges
DROP_BUDGET = 4e-3             # allowed relative Frobenius perturbation of V

_CACHE = {}


def _build_V(angles, Bmat):
    """V = U_38 @ ... @ U_0 in float64, where U_g = cos(th) A + sin(th) B + C.

    B[g, j, i] == +1 identifies the coupled pair (i, j): U[i,i]=U[j,j]=cos,
    U[j,i]=+sin, U[i,j]=-sin; all other rows are identity.
    """
    V = np.eye(D, dtype=np.float64)
    for g in range(N_GATES):
        jj, ii = np.nonzero(Bmat[g] > 0.5)
        c = np.cos(float(angles[g]))
        s = np.sin(float(angles[g]))
        Vi = V[ii, :].copy()
        Vj = V[jj, :].copy()
        V[ii, :] = c * Vi - s * Vj
        V[jj, :] = s * Vi + c * Vj
    return V


def _plan_intervals(V):
    """Per (k-tile, PSUM bank): [c0, c1) column interval of V^T holding all
    significant mass, or None.
    """
    VT = V.T  # [k, n] — the rhs layout
    sliver = np.zeros((KT, D))
    for kc, (k0, ksz) in enumerate(CHUNKS):
        sliver[kc] = (VT[k0:k0 + ksz, :] ** 2).sum(axis=0)
    tot = sliver.sum()
    flat = np.sort(sliver.ravel())
    csum = np.cumsum(flat)
    budget = DROP_BUDGET ** 2 * tot
    pos = np.searchsorted(csum, budget)
    thr = flat[pos - 1] if pos > 0 else -1.0
    sig = sliver > thr

    intervals = []  # [kc][bank] -> (c0, c1) or None
    for kc in range(KT):
        row = []
        for b0, b1 in BANKS:
            cols = np.nonzero(sig[kc, b0:b1])[0]
            if len(cols) == 0:
                row.append(None)
                continue
            c0 = int(b0 + cols[0]) & ~1          # 8-byte-align start
            c1 = min(b1, (int(b0 + cols[-1]) + 2) & ~1)
            row.append((c0, c1))
        intervals.append(row)

    # safety: every column must be covered by at least one kept interval,
    # else the PSUM evacuation would read stale garbage there.
    covered = np.zeros(D, bool)
    for row in intervals:
        for iv in row:
            if iv is not None:
                covered[iv[0]:iv[1]] = True
    if not covered.all():
        for bi, (b0, b1) in enumerate(BANKS):
            if not covered[b0:b1].all():
                kc = int(sliver[:, b0:b1].sum(axis=1).argmax())
                intervals[kc][bi] = (b0, b1)
    return intervals


def _build_program(intervals):
    import concourse.bacc as bacc
    import concourse.mybir as mybir
    import concourse.tile as tile

    nc = bacc.Bacc("TRN2", target_bir_lowering=False, debug=False,
                   num_devices=N_CORES)
    bf16 = mybir.dt.bfloat16
    f32 = mybir.dt.float32

    # host-packed X: x1[b, p, kc, n] = X[b, kc*128+p, n] for kc<6 (so each
    # partition's slice is 9360 B contiguous), x2[b] = rows 768:780.
    x1 = nc.dram_tensor("x1", [B_LOC, P, KT - 1, D], bf16,
                        kind="ExternalInput").ap()
    x2 = nc.dram_tensor("x2", [B_LOC, LAST, D], bf16,
                        kind="ExternalInput").ap()
    vt1 = nc.dram_tensor("vt1", [P, KT - 1, D], bf16,
                         kind="ExternalInput").ap()
    vt2 = nc.dram_tensor("vt2", [LAST, D], bf16,
                         kind="ExternalInput").ap()
    # packed output: chunk pairs (0,1),(2,3),(4,5) -> y1[b, jc, p, t, n] holds
    # row 256*jc + 128*t + p; the 12-row tail goes to y2. Host unpacks.
    y1 = nc.dram_tensor("y1", [B_LOC, 3, P, 2, D], bf16,
                        kind="ExternalOutput").ap()
    y2 = nc.dram_tensor("y2", [B_LOC, LAST, D], bf16,
                        kind="ExternalOutput").ap()

    # flat list of kept (kc, bank_idx, c0, c1) in kc order with bank pairs
    # adjacent (so the duplicate-LDWEIGHTS dedupe can fire on the pairs).
    kc_order = list(range(KT))
    kept = [(kc, bi, iv[0], iv[1])
            for kc in kc_order for bi, iv in enumerate(intervals[kc])
            if iv is not None]
    first_kc = {}
    last_kc = {}
    for kc, bi, _, _ in kept:
        first_kc.setdefault(bi, kc)
        last_kc[bi] = kc

    BK0 = 512                       # PSUM bank boundary in fp32 cols

    with tile.TileContext(nc) as tc:
        with (
            tc.tile_pool(name="vtp", bufs=1) as vtp,
            tc.tile_pool(name="xb", bufs=3) as xbp,
            tc.tile_pool(name="pt", bufs=2) as ptp,
            tc.tile_pool(name="yo", bufs=3) as yop,
            tc.tile_pool(name="wup", bufs=1) as wup,
            tc.tile_pool(name="ps", bufs=4, space="PSUM") as psp,
        ):
            # PE warmup: a few dummy matmuls start the HAM clock ramp while
            # the first DMAs land; they use psum-pool generations that
            # rotate away before real work needs them.
            wz = wup.tile([P, 256], bf16)
            nc.vector.memset(wz[:], 0.0)
            ps_w = psp.tile([P, D], f32, tag="ps")
            for _ in range(6):
                nc.tensor.matmul(ps_w[:, :256], wz[:, :P], wz[:, :256],
                                 start=True, stop=True)

            # V^T resident in SBUF, k-partitioned: vt_sb[p, kc, n]. The last
            # k-chunk is zero-padded to K=128: a K=12 matmul (32-row tile
            # size) costs a ~100 ns array drain on itself AND its successor,
            # so full-K with zeroed pads is much cheaper.
            vt_sb = vtp.tile([P, KT, D], bf16)
            nc.any.memzero(vt_sb[:, KT - 1, :])

            grp_idx = 0

            def evac(out_ap, psum_ap, parity):
                # rotate PSUM evacuation between ScalarE and VectorE so each
                # engine alternates the wide (512-col) and narrow (268-col)
                # bank copies across groups
                if parity % 2 == 0:
                    nc.scalar.copy(out_ap, psum_ap)
                else:
                    nc.vector.tensor_copy(out=out_ap, in_=psum_ap)

            def pass_mms(ps, src_fn, msz):
                for kc, bi, c0, c1 in kept:
                    nc.tensor.matmul(
                        ps[:msz, c0:c1],
                        src_fn(kc),
                        vt_sb[:, kc, c0:c1],
                        start=(kc == first_kc[bi]),
                        stop=(kc == last_kc[bi]),
                    )

            def load_x(b):
                xb = xbp.tile([P, KT, D], bf16, tag="xb")
                nc.any.memzero(xb[:, KT - 1, :])
                nc.sync.dma_start(xb[:, : KT - 1, :], x1[b])
                nc.sync.dma_start(xb[:LAST, KT - 1, :], x2[b])
                return xb

            def load_x0():
                # startup: load V^T and X0 chunk-by-chunk in matmul emission
                # order so the first group starts (subtile deps) after ~0.4 MB
                # instead of after all of V^T + X0.
                xb = xbp.tile([P, KT, D], bf16, tag="xb")
                nc.any.memzero(xb[:, KT - 1, :])
                for kc in kc_order:
                    if kc == KT - 1:
                        nc.sync.dma_start(vt_sb[:LAST, kc, :], vt2)
                        nc.sync.dma_start(xb[:LAST, kc, :], x2[0])
                    else:
                        nc.sync.dma_start(vt_sb[:, kc, :], vt1[:, kc, :])
                        nc.sync.dma_start(xb[:, kc, :], x1[0, :, kc, :])
                return xb

            EV0 = 392     # balanced evac split point (copies may span banks)

            def emit_p1(mc, xb, pch):
                nonlocal grp_idx
                m0, msz = CHUNKS[mc]
                ps = psp.tile([P, D], f32, tag="ps")
                pass_mms(ps, lambda kc: xb[:, kc, m0:m0 + msz], msz)
                evac(pch[mc][:msz, :EV0], ps[:msz, :EV0], grp_idx)
                evac(pch[mc][:msz, EV0:], ps[:msz, EV0:], grp_idx + 1)
                grp_idx += 1

            def emit_p2(b, ic, pch, yo_pair):
                nonlocal grp_idx
                i0, isz = CHUNKS[ic]
                ps = psp.tile([P, D], f32, tag="ps")
                pass_mms(ps, lambda kc: pch[kc][:, i0:i0 + isz], isz)
                if ic == KT - 1:
                    yo = yop.tile([LAST, D], bf16, tag="yot")
                    evac(yo[:isz, :EV0], ps[:isz, :EV0], grp_idx)
                    evac(yo[:isz, EV0:], ps[:isz, EV0:], grp_idx + 1)
                    grp_idx += 1
                    nc.sync.dma_start(y2[b], yo[:isz, :])
                    return None
                if yo_pair is None:
                    yo_pair = yop.tile([P, 2, D], bf16, tag="yo")
                t = ic % 2
                evac(yo_pair[:isz, t, :EV0], ps[:isz, :EV0], grp_idx)
                evac(yo_pair[:isz, t, EV0:], ps[:isz, EV0:], grp_idx + 1)
                grp_idx += 1
                if t == 1:
                    nc.sync.dma_start(y1[b, ic // 2], yo_pair[:])
                    return None
                return yo_pair

            # software pipeline: slot b runs pass-1 of batch b interleaved
            # with pass-2 of batch b-1. Pass-2 chunks trail by 2 so the last
            # pass-1 evacuations of batch b-1 have landed before pass-2
            # needs pchunk[5] and [6].
            ORDER = [("p1", 0), ("p1", 1)]
            for i in range(2, KT):
                ORDER += [("p1", i), ("p2", i - 2)]
            ORDER += [("p2", KT - 2), ("p2", KT - 1)]

            xb_cur = load_x0()
            pch_prev = None
            yo_pair = None
            for b in range(B_LOC):
                pch_cur = [ptp.tile([P, D], bf16, tag=f"pt{i}", name=f"pc{i}")
                           for i in range(KT)]
                nc.any.memzero(pch_cur[KT - 1][:])
                if b + 1 < B_LOC:
                    xb_next = load_x(b + 1)
                for kind, i in ORDER:
                    if kind == "p1":
                        emit_p1(i, xb_cur, pch_cur)
                    elif pch_prev is not None:
                        yo_pair = emit_p2(b - 1, i, pch_prev, yo_pair)
                pch_prev = pch_cur
                xb_cur = xb_next
            for i in range(KT):
                yo_pair = emit_p2(B_LOC - 1, i, pch_prev, yo_pair)

    nc.compile()
    _dedupe_ldweights(nc)
    return nc


def _dedupe_ldweights(nc):
    """Drop an InstLdweights whose weights AP is identical to the previous
    one with only PE matmuls in between — the weights are already resident
    in the PE array, so the reload only burns weight-port bandwidth (the
    group bottleneck). Only sync-free loads are dropped, and matmul
    semaphore updates are untouched, so the schedule's counts are
    preserved. Runs post-compile, pre-serialization.
    """
    import concourse.mybir as mybir

    removed = 0
    for blk in nc.main_func.blocks:
        insts = blk.instructions
        last_key = None
        drop = []
        for x in insts:
            if isinstance(x, mybir.InstLdweights):
                si = x.sync_info
                clean = si is None or (len(si.on_wait) == 0
                                       and len(si.on_update) == 0)
                key = str(x.ins[0])
                if clean and key == last_key:
                    drop.append(x)
                    continue
                last_key = key
            elif not isinstance(x, mybir.InstMatmult):
                # conservatively assume anything else on the PE engine (or
                # control flow) may disturb the loaded weights
                eng = getattr(x, "engine", None)
                if eng is None or "PE" in str(eng):
                    last_key = None
        for x in drop:
            insts.remove(x)
        removed += len(drop)
    return removed


def _get_program(intervals):
    key = tuple(tuple(row) for row in intervals)
    if _CACHE.get("key") != key:
        _CACHE["nc"] = _build_program(intervals)
        _CACHE["key"] = key
    return _CACHE["nc"]


def kernel(input_state, angles, A, B, C, _trace=False):
    from concourse.bass_utils import run_bass_kernel_spmd

    X = np.asarray(input_state, dtype=np.float32)
    V = _build_V(np.asarray(angles, dtype=np.float64), np.asarray(B))
    vt = np.ascontiguousarray(V.T).astype(ml_dtypes.bfloat16)
    X_bf = X.astype(ml_dtypes.bfloat16)

    # pack: x1[b, p, kc, n] = X[b, kc*128+p, n]
    x1 = np.ascontiguousarray(
        X_bf[:, :FULL].reshape(B_TOTAL, KT - 1, P, D).transpose(0, 2, 1, 3))
    x2 = np.ascontiguousarray(X_bf[:, FULL:])
    vt1 = np.ascontiguousarray(
        vt[:FULL].reshape(KT - 1, P, D).transpose(1, 0, 2))
    vt2 = np.ascontiguousarray(vt[FULL:])

    intervals = _plan_intervals(V)
    nc = _get_program(intervals)
    in_maps = [
        {"x1": x1[c * B_LOC:(c + 1) * B_LOC],
         "x2": x2[c * B_LOC:(c + 1) * B_LOC],
         "vt1": vt1, "vt2": vt2}
        for c in range(N_CORES)
    ]
    res = run_bass_kernel_spmd(nc, in_maps, core_ids=list(range(N_CORES)),
                               trace=_trace)
    out = np.empty((B_TOTAL, D, D), np.float32)
    for c in range(N_CORES):
        # y1[b, jc, p, t, n] holds row 256*jc + 128*t + p; y2 holds 768:780
        y1 = np.asarray(res.results[c]["y1"], dtype=np.float32)
        y2 = np.asarray(res.results[c]["y2"], dtype=np.float32)
        sl = out[c * B_LOC:(c + 1) * B_LOC]
        sl[:, :FULL] = y1.transpose(0, 1, 3, 2, 4).reshape(B_LOC, FULL, D)
        sl[:, FULL:] = y2
    if _trace:
        kernel.last_results = res
    return out


# revision 31
# speedup vs baseline: 1.1775x; 1.0020x over previous
"""Trainium2 kernel for nn_Dense_RBS_density_3D.

The reference applies 39 RBS gates sequentially to a batch of 64 density
matrices: rho <- U_g rho U_g^T. The gates compose, so the whole circuit is a
single orthogonal matrix V = U_38 @ ... @ U_0 (depends only on the 39 scalar
angles + the fixed sparsity structure), and the output is V @ rho @ V^T per
batch element.

Host side: build V from the angles (39 sparse pair-rotation sweeps applied to
an identity matrix). V inherits strong structural sparsity with geometric
magnitude decay from the adjacent-qubit gate ladder. The host computes, per
(contraction-tile, PSUM-bank), the column interval of V^T that carries
significant mass; everything outside is skipped on device (perturbs V by
~4e-3 in Frobenius norm — the bf16 noise floor is ~3.8e-3 and the gate 2e-2).

Device side (8 NeuronCores, data-parallel over batch): per batch element
compute Y = V X V^T as two transpose-free matmul passes of the same shape:

    f(Z) = Z^T @ V^T   (lhsT = Z with contraction on partitions, rhs = V^T)
    Y = f(f(X))        since (X^T V^T)^T V^T = V X V^T

bf16 operands (X pre-cast + pre-packed on host) with fp32 PSUM accumulation.
Pass 2 of batch b-1 is software-pipelined against pass 1 of batch b at
m-chunk granularity so PSUM-evacuation latency and the "pass 2 needs all 7
pchunks" dependency never stall the PE. X is packed on host into a
[128, 6, 780] per-batch layout so each load descriptor is 9.4 KB/partition
instead of a 1.5 KB row. The ragged last k-chunk (12 rows) runs as K=12
matmuls — no zero padding or memzero needed anywhere.
"""

import numpy as np
import ml_dtypes

D = 780           # binom(40, 2)
N_GATES = 39
B_TOTAL = 64
N_CORES = 8
B_LOC = B_TOTAL // N_CORES
P = 128
KT = (D + P - 1) // P          # 7 k-chunks: 6x128 + 12
LAST = D - (KT - 1) * P        # 12
FULL = (KT - 1) * P            # 768
CHUNKS = [(i * P, min(P, D - i * P)) for i in range(KT)]
BANKS = [(0, 512), (512, D)]   # PSUM fp32 bank col ranges
DROP_BUDGET = 4e-3             # allowed relative Frobenius perturbation of V

_CACHE = {}


def _build_V(angles, Bmat):
    """V = U_38 @ ... @ U_0 in float64, where U_g = cos(th) A + sin(th) B + C.

    B[g, j, i] == +1 identifies the coupled pair (i, j): U[i,i]=U[j,j]=cos,
    U[j,i]=+sin, U[i,j]=-sin; all other rows are identity.
    """
    V = np.eye(D, dtype=np.float64)
    for g in range(N_GATES):
        jj, ii = np.nonzero(Bmat[g] > 0.5)
        c = np.cos(float(angles[g]))
        s = np.sin(float(angles[g]))
        Vi = V[ii, :].copy()
        Vj = V[jj, :].copy()
        V[ii, :] = c * Vi - s * Vj
        V[jj, :] = s * Vi + c * Vj
    return V


def _plan_intervals(V):
    """Per (k-tile, PSUM bank): [c0, c1) column interval of V^T holding all
    significant mass, or None.
    """
    VT = V.T  # [k, n] — the rhs layout
    sliver = np.zeros((KT, D))
    for kc, (k0, ksz) in enumerate(CHUNKS):
        sliver[kc] = (VT[k0:k0 + ksz, :] ** 2).sum(axis=0)
    tot = sliver.sum()
    flat = np.sort(sliver.ravel())
    csum = np.cumsum(flat)
    budget = DROP_BUDGET ** 2 * tot
    pos = np.searchsorted(csum, budget)
    thr = flat[pos - 1] if pos > 0 else -1.0
    sig = sliver > thr

    intervals = []  # [kc][bank] -> (c0, c1) or None
    for kc in range(KT):
        row = []
        for b0, b1 in BANKS:
            cols = np.nonzero(sig[kc, b0:b1])[0]
            if len(cols) == 0:
                row.append(None)
                continue
            c0 = int(b0 + cols[0]) & ~1          # 8-byte-align start
            c1 = min(b1, (int(b0 + cols[-1]) + 2) & ~1)
            row.append((c0, c1))
        intervals.append(row)

    # safety: every column must be covered by at least one kept interval,
    # else the PSUM evacuation would read stale garbage there.
    covered = np.zeros(D, bool)
    for row in intervals:
        for iv in row:
            if iv is not None:
                covered[iv[0]:iv[1]] = True
    if not covered.all():
        for bi, (b0, b1) in enumerate(BANKS):
            if not covered[b0:b1].all():
                kc = int(sliver[:, b0:b1].sum(axis=1).argmax())
                intervals[kc][bi] = (b0, b1)
    return intervals


def _build_program(intervals):
    import concourse.bacc as bacc
    import concourse.mybir as mybir
    import concourse.tile as tile

    nc = bacc.Bacc("TRN2", target_bir_lowering=False, debug=False,
                   num_devices=N_CORES)
    bf16 = mybir.dt.bfloat16
    f32 = mybir.dt.float32

    # host-packed X: x1[b, p, kc, n] = X[b, kc*128+p, n] for kc<6 (so each
    # partition's slice is 9360 B contiguous), x2[b] = rows 768:780.
    x1 = nc.dram_tensor("x1", [B_LOC, P, KT - 1, D], bf16,
                        kind="ExternalInput").ap()
    x2 = nc.dram_tensor("x2", [B_LOC, LAST, D], bf16,
                        kind="ExternalInput").ap()
    vt1 = nc.dram_tensor("vt1", [P, KT - 1, D], bf16,
                         kind="ExternalInput").ap()
    vt2 = nc.dram_tensor("vt2", [LAST, D], bf16,
                         kind="ExternalInput").ap()
    # pass-2 runs over the flat (batch, row) axis: 6240 rows in 49 chunks.
    # Output chunk pairs pack into y1[jc, p, t, n] = flat row 256*jc+128*t+p;
    # the final 96-row chunk goes to y2. Host unpacks (flat row = b*780 + c).
    NJ2 = (B_LOC * D + P - 1) // P           # 49
    CH2 = [(j * P, min(P, B_LOC * D - j * P)) for j in range(NJ2)]
    y1 = nc.dram_tensor("y1", [NJ2 // 2, P, 2, D], bf16,
                        kind="ExternalOutput").ap()
    y2 = nc.dram_tensor("y2", [CH2[-1][1], D], bf16,
                        kind="ExternalOutput").ap()

    # flat list of kept (kc, bank_idx, c0, c1) in kc order with bank pairs
    # adjacent (so the duplicate-LDWEIGHTS dedupe can fire on the pairs).
    kc_order = list(range(KT))
    kept = [(kc, bi, iv[0], iv[1])
            for kc in kc_order for bi, iv in enumerate(intervals[kc])
            if iv is not None]
    first_kc = {}
    last_kc = {}
    for kc, bi, _, _ in kept:
        first_kc.setdefault(bi, kc)
        last_kc[bi] = kc

    BK0 = 512                       # PSUM bank boundary in fp32 cols

    with tile.TileContext(nc) as tc:
        with (
            tc.tile_pool(name="vtp", bufs=1) as vtp,
            tc.tile_pool(name="xb", bufs=3) as xbp,
            tc.tile_pool(name="pt", bufs=1) as ptp,
            tc.tile_pool(name="yo", bufs=3) as yop,
            tc.tile_pool(name="wup", bufs=1) as wup,
            tc.tile_pool(name="ps", bufs=4, space="PSUM") as psp,
        ):
            # PE warmup: a few dummy matmuls start the HAM clock ramp while
            # the first DMAs land; they use psum-pool generations that
            # rotate away before real work needs them.
            wz = wup.tile([P, 256], bf16)
            nc.vector.memset(wz[:], 0.0)
            ps_w = psp.tile([P, D], f32, tag="ps")
            for _ in range(6):
                nc.tensor.matmul(ps_w[:, :256], wz[:, :P], wz[:, :256],
                                 start=True, stop=True)

            # V^T resident in SBUF, k-partitioned: vt_sb[p, kc, n]. The last
            # k-chunk is zero-padded to K=128: a K=12 matmul (32-row tile
            # size) costs a ~100 ns array drain on itself AND its successor,
            # so full-K with zeroed pads is much cheaper.
            vt_sb = vtp.tile([P, KT, D], bf16)
            nc.any.memzero(vt_sb[:, KT - 1, :])

            grp_idx = 0

            def evac(out_ap, psum_ap, parity):
                # rotate PSUM evacuation between ScalarE and VectorE so each
                # engine alternates the wide (512-col) and narrow (268-col)
                # bank copies across groups
                if parity % 2 == 0:
                    nc.scalar.copy(out_ap, psum_ap)
                else:
                    nc.vector.tensor_copy(out=out_ap, in_=psum_ap)

            def pass_mms(ps, src_fn, msz):
                for kc, bi, c0, c1 in kept:
                    nc.tensor.matmul(
                        ps[:msz, c0:c1],
                        src_fn(kc),
                        vt_sb[:, kc, c0:c1],
                        start=(kc == first_kc[bi]),
                        stop=(kc == last_kc[bi]),
                    )

            def load_x(b):
                xb = xbp.tile([P, KT, D], bf16, tag="xb")
                nc.any.memzero(xb[:, KT - 1, :])
                nc.sync.dma_start(xb[:, : KT - 1, :], x1[b])
                nc.sync.dma_start(xb[:LAST, KT - 1, :], x2[b])
                return xb

            def load_x0():
                # startup: load V^T and X0 chunk-by-chunk in matmul emission
                # order so the first group starts (subtile deps) after ~0.4 MB
                # instead of after all of V^T + X0.
                xb = xbp.tile([P, KT, D], bf16, tag="xb")
                nc.any.memzero(xb[:, KT - 1, :])
                for kc in kc_order:
                    if kc == KT - 1:
                        nc.sync.dma_start(vt_sb[:LAST, kc, :], vt2)
                        nc.sync.dma_start(xb[:LAST, kc, :], x2[0])
                    else:
                        nc.sync.dma_start(vt_sb[:, kc, :], vt1[:, kc, :])
                        nc.sync.dma_start(xb[:, kc, :], x1[0, :, kc, :])
                return xb

            EV0 = 392     # balanced evac split point (copies may span banks)

            # pchunks: single-generation tiles holding PT for ALL batches,
            # pch[kc][p, b*780 + c] = PT_b[kc*128 + p, c]. Pass-2 then runs
            # over the flat 6240-row axis in 49 full-width chunks (no ragged
            # 12-row pass-2 groups). The kc=6 pad partitions are zeroed once.
            pch = [ptp.tile([P, B_LOC * D], bf16, tag=f"pt{i}", name=f"pc{i}")
                   for i in range(KT)]
            nc.any.memzero(pch[KT - 1][:])

            def emit_p1(b, mc, xb):
                nonlocal grp_idx
                m0, msz = CHUNKS[mc]
                ps = psp.tile([P, D], f32, tag="ps")
                pass_mms(ps, lambda kc: xb[:, kc, m0:m0 + msz], msz)
                dst = pch[mc][:, b * D:(b + 1) * D]
                evac(dst[:msz, :EV0], ps[:msz, :EV0], grp_idx)
                evac(dst[:msz, EV0:], ps[:msz, EV0:], grp_idx + 1)
                grp_idx += 1

            def emit_p2(j, yo_pair):
                nonlocal grp_idx
                j0, jsz = CH2[j]
                ps = psp.tile([P, D], f32, tag="ps")
                pass_mms(ps, lambda kc: pch[kc][:, j0:j0 + jsz], jsz)
                if j == NJ2 - 1:
                    yo = yop.tile([P, D], bf16, tag="yot")
                    evac(yo[:jsz, :EV0], ps[:jsz, :EV0], grp_idx)
                    evac(yo[:jsz, EV0:], ps[:jsz, EV0:], grp_idx + 1)
                    grp_idx += 1
                    nc.sync.dma_start(y2[:], yo[:jsz, :])
                    return None
                if yo_pair is None:
                    yo_pair = yop.tile([P, 2, D], bf16, tag="yo")
                t = j % 2
                evac(yo_pair[:jsz, t, :EV0], ps[:jsz, :EV0], grp_idx)
                evac(yo_pair[:jsz, t, EV0:], ps[:jsz, EV0:], grp_idx + 1)
                grp_idx += 1
                if t == 1:
                    nc.sync.dma_start(y1[j // 2], yo_pair[:])
                    return None
                return yo_pair

            # software pipeline: slot b runs pass-1 of batch b interleaved
            # with the pass-2 flat chunks that became computable after batch
            # b-1 (those reading columns < 780*b). The first pass-2 chunk of
            # a slot trails two pass-1 groups so batch b-1's last
            # evacuations have landed.
            xb_cur = load_x0()
            yo_pair = None
            q = 0
            for b in range(B_LOC):
                if b + 1 < B_LOC:
                    xb_next = load_x(b + 1)
                avail = (D * b) // P
                for i in range(KT):
                    emit_p1(b, i, xb_cur)
                    if i >= 1 and q < avail:
                        yo_pair = emit_p2(q, yo_pair)
                        q += 1
                while q < avail:
                    yo_pair = emit_p2(q, yo_pair)
                    q += 1
                xb_cur = xb_next
            while q < NJ2:
                yo_pair = emit_p2(q, yo_pair)
                q += 1

    nc.compile()
    _dedupe_ldweights(nc)
    return nc


def _dedupe_ldweights(nc):
    """Drop an InstLdweights whose weights AP is identical to the previous
    one with only PE matmuls in between — the weights are already resident
    in the PE array, so the reload only burns weight-port bandwidth (the
    group bottleneck). Only sync-free loads are dropped, and matmul
    semaphore updates are untouched, so the schedule's counts are
    preserved. Runs post-compile, pre-serialization.
    """
    import concourse.mybir as mybir

    removed = 0
    for blk in nc.main_func.blocks:
        insts = blk.instructions
        last_key = None
        drop = []
        for x in insts:
            if isinstance(x, mybir.InstLdweights):
                si = x.sync_info
                clean = si is None or (len(si.on_wait) == 0
                                       and len(si.on_update) == 0)
                key = str(x.ins[0])
                if clean and key == last_key:
                    drop.append(x)
                    continue
                last_key = key
            elif not isinstance(x, mybir.InstMatmult):
                # conservatively assume anything else on the PE engine (or
                # control flow) may disturb the loaded weights
                eng = getattr(x, "engine", None)
                if eng is None or "PE" in str(eng):
                    last_key = None
        for x in drop:
            insts.remove(x)
        removed += len(drop)
    return removed


def _get_program(intervals):
    key = tuple(tuple(row) for row in intervals)
    if _CACHE.get("key") != key:
        _CACHE["nc"] = _build_program(intervals)
        _CACHE["key"] = key
    return _CACHE["nc"]


def kernel(input_state, angles, A, B, C, _trace=False):
    from concourse.bass_utils import run_bass_kernel_spmd

    X = np.asarray(input_state, dtype=np.float32)
    V = _build_V(np.asarray(angles, dtype=np.float64), np.asarray(B))
    vt = np.ascontiguousarray(V.T).astype(ml_dtypes.bfloat16)
    X_bf = X.astype(ml_dtypes.bfloat16)

    # pack: x1[b, p, kc, n] = X[b, kc*128+p, n]
    x1 = np.ascontiguousarray(
        X_bf[:, :FULL].reshape(B_TOTAL, KT - 1, P, D).transpose(0, 2, 1, 3))
    x2 = np.ascontiguousarray(X_bf[:, FULL:])
    vt1 = np.ascontiguousarray(
        vt[:FULL].reshape(KT - 1, P, D).transpose(1, 0, 2))
    vt2 = np.ascontiguousarray(vt[FULL:])

    intervals = _plan_intervals(V)
    nc = _get_program(intervals)
    in_maps = [
        {"x1": x1[c * B_LOC:(c + 1) * B_LOC],
         "x2": x2[c * B_LOC:(c + 1) * B_LOC],
         "vt1": vt1, "vt2": vt2}
        for c in range(N_CORES)
    ]
    res = run_bass_kernel_spmd(nc, in_maps, core_ids=list(range(N_CORES)),
                               trace=_trace)
    out = np.empty((B_TOTAL, D, D), np.float32)
    n_pairs = (B_LOC * D) // (2 * P)        # 24
    for c in range(N_CORES):
        # y1[jc, p, t, n] = flat row 256*jc + 128*t + p; y2 = final 96 rows;
        # flat row = b*780 + c within the core's 8 batches
        y1 = np.asarray(res.results[c]["y1"], dtype=np.float32)
        y2 = np.asarray(res.results[c]["y2"], dtype=np.float32)
        flat = np.empty((B_LOC * D, D), np.float32)
        flat[:n_pairs * 2 * P] = y1.transpose(0, 2, 1, 3).reshape(-1, D)
        flat[n_pairs * 2 * P:] = y2
        out[c * B_LOC:(c + 1) * B_LOC] = flat.reshape(B_LOC, D, D)
    if _trace:
        kernel.last_results = res
    return out


# revision 36
# speedup vs baseline: 1.2304x; 1.0449x over previous
"""Trainium2 kernel for nn_Dense_RBS_density_3D.

The reference applies 39 RBS gates sequentially to a batch of 64 density
matrices: rho <- U_g rho U_g^T. The gates compose, so the whole circuit is a
single orthogonal matrix V = U_38 @ ... @ U_0 (depends only on the 39 scalar
angles + the fixed sparsity structure), and the output is V @ rho @ V^T per
batch element.

Host side: build V from the angles (39 sparse pair-rotation sweeps applied to
an identity matrix). V inherits strong structural sparsity with geometric
magnitude decay from the adjacent-qubit gate ladder. The host computes, per
(contraction-tile, PSUM-bank), the column interval of V^T that carries
significant mass; everything outside is skipped on device (perturbs V by
~4e-3 in Frobenius norm — the bf16 noise floor is ~3.8e-3 and the gate 2e-2).

Device side (8 NeuronCores, data-parallel over batch): per batch element
compute Y = V X V^T as two transpose-free matmul passes of the same shape:

    f(Z) = Z^T @ V^T   (lhsT = Z with contraction on partitions, rhs = V^T)
    Y = f(f(X))        since (X^T V^T)^T V^T = V X V^T

bf16 operands (X pre-cast + pre-packed on host) with fp32 PSUM accumulation.
Pass 2 of batch b-1 is software-pipelined against pass 1 of batch b at
m-chunk granularity so PSUM-evacuation latency and the "pass 2 needs all 7
pchunks" dependency never stall the PE. X is packed on host into a
[128, 6, 780] per-batch layout so each load descriptor is 9.4 KB/partition
instead of a 1.5 KB row. The ragged last k-chunk (12 rows) runs as K=12
matmuls — no zero padding or memzero needed anywhere.
"""

import numpy as np
import ml_dtypes

D = 780           # binom(40, 2)
N_GATES = 39
B_TOTAL = 64
N_CORES = 8
B_LOC = B_TOTAL // N_CORES
P = 128
KT = (D + P - 1) // P          # 7 k-chunks: 6x128 + 12
LAST = D - (KT - 1) * P        # 12
FULL = (KT - 1) * P            # 768
CHUNKS = [(i * P, min(P, D - i * P)) for i in range(KT)]
BANKS = [(0, 512), (512, D)]   # PSUM fp32 bank col ranges
DROP_BUDGET = 4e-3             # allowed relative Frobenius perturbation of V

_CACHE = {}


def _build_V(angles, Bmat):
    """V = U_38 @ ... @ U_0 in float64, where U_g = cos(th) A + sin(th) B + C.

    B[g, j, i] == +1 identifies the coupled pair (i, j): U[i,i]=U[j,j]=cos,
    U[j,i]=+sin, U[i,j]=-sin; all other rows are identity.
    """
    V = np.eye(D, dtype=np.float64)
    for g in range(N_GATES):
        jj, ii = np.nonzero(Bmat[g] > 0.5)
        c = np.cos(float(angles[g]))
        s = np.sin(float(angles[g]))
        Vi = V[ii, :].copy()
        Vj = V[jj, :].copy()
        V[ii, :] = c * Vi - s * Vj
        V[jj, :] = s * Vi + c * Vj
    return V


def _plan_intervals(V):
    """Per (k-tile, PSUM bank): [c0, c1) column interval of V^T holding all
    significant mass, or None.
    """
    VT = V.T  # [k, n] — the rhs layout
    sliver = np.zeros((KT, D))
    for kc, (k0, ksz) in enumerate(CHUNKS):
        sliver[kc] = (VT[k0:k0 + ksz, :] ** 2).sum(axis=0)
    tot = sliver.sum()
    flat = np.sort(sliver.ravel())
    csum = np.cumsum(flat)
    budget = DROP_BUDGET ** 2 * tot
    pos = np.searchsorted(csum, budget)
    thr = flat[pos - 1] if pos > 0 else -1.0
    sig = sliver > thr

    intervals = []  # [kc][bank] -> (c0, c1) or None
    for kc in range(KT):
        row = []
        for b0, b1 in BANKS:
            cols = np.nonzero(sig[kc, b0:b1])[0]
            if len(cols) == 0:
                row.append(None)
                continue
            c0 = int(b0 + cols[0]) & ~1          # 8-byte-align start
            c1 = min(b1, (int(b0 + cols[-1]) + 2) & ~1)
            row.append((c0, c1))
        intervals.append(row)

    # safety: every column must be covered by at least one kept interval,
    # else the PSUM evacuation would read stale garbage there.
    covered = np.zeros(D, bool)
    for row in intervals:
        for iv in row:
            if iv is not None:
                covered[iv[0]:iv[1]] = True
    if not covered.all():
        for bi, (b0, b1) in enumerate(BANKS):
            if not covered[b0:b1].all():
                kc = int(sliver[:, b0:b1].sum(axis=1).argmax())
                intervals[kc][bi] = (b0, b1)
    return intervals


def _build_program(intervals):
    import concourse.bacc as bacc
    import concourse.mybir as mybir
    import concourse.tile as tile

    nc = bacc.Bacc("TRN2", target_bir_lowering=False, debug=False,
                   num_devices=N_CORES)
    bf16 = mybir.dt.bfloat16
    f32 = mybir.dt.float32

    # host-packed X: x1[b, p, kc, n] = X[b, kc*128+p, n] for kc<6 (so each
    # partition's slice is 9360 B contiguous), x2[b] = rows 768:780.
    x1 = nc.dram_tensor("x1", [B_LOC, P, KT - 1, D], bf16,
                        kind="ExternalInput").ap()
    x2 = nc.dram_tensor("x2", [B_LOC, LAST, D], bf16,
                        kind="ExternalInput").ap()
    vt1 = nc.dram_tensor("vt1", [P, KT - 1, D], bf16,
                         kind="ExternalInput").ap()
    vt2 = nc.dram_tensor("vt2", [LAST, D], bf16,
                         kind="ExternalInput").ap()
    # pass-2 runs over the flat (batch, row) axis: 6240 rows in 49 chunks.
    # Output chunk pairs pack into y1[jc, p, t, n] = flat row 256*jc+128*t+p;
    # the final 96-row chunk goes to y2. Host unpacks (flat row = b*780 + c).
    NJ2 = (B_LOC * D + P - 1) // P           # 49
    CH2 = [(j * P, min(P, B_LOC * D - j * P)) for j in range(NJ2)]
    y1 = nc.dram_tensor("y1", [NJ2 // 4, P, 4, D], bf16,
                        kind="ExternalOutput").ap()
    y2 = nc.dram_tensor("y2", [CH2[-1][1], D], bf16,
                        kind="ExternalOutput").ap()

    # flat list of kept (kc, bank_idx, c0, c1) in kc order with bank pairs
    # adjacent (so the duplicate-LDWEIGHTS dedupe can fire on the pairs).
    kc_order = list(range(KT))
    kept = [(kc, bi, iv[0], iv[1])
            for kc in kc_order for bi, iv in enumerate(intervals[kc])
            if iv is not None]
    first_kc = {}
    last_kc = {}
    for kc, bi, _, _ in kept:
        first_kc.setdefault(bi, kc)
        last_kc[bi] = kc

    BK0 = 512                       # PSUM bank boundary in fp32 cols

    with tile.TileContext(nc) as tc:
        with (
            tc.tile_pool(name="vtp", bufs=1) as vtp,
            tc.tile_pool(name="xb", bufs=3) as xbp,
            tc.tile_pool(name="pt", bufs=1) as ptp,
            tc.tile_pool(name="yo", bufs=3) as yop,
            tc.tile_pool(name="wup", bufs=1) as wup,
            tc.tile_pool(name="ps", bufs=4, space="PSUM") as psp,
        ):
            # PE warmup: a few dummy matmuls start the HAM clock ramp while
            # the first DMAs land; they use psum-pool generations that
            # rotate away before real work needs them.
            wz = wup.tile([P, 256], bf16)
            nc.vector.memset(wz[:], 0.0)
            ps_w = psp.tile([P, D], f32, tag="ps")
            for _ in range(6):
                nc.tensor.matmul(ps_w[:, :256], wz[:, :P], wz[:, :256],
                                 start=True, stop=True)

            # V^T resident in SBUF, k-partitioned: vt_sb[p, kc, n]. The last
            # k-chunk is zero-padded to K=128: a K=12 matmul (32-row tile
            # size) costs a ~100 ns array drain on itself AND its successor,
            # so full-K with zeroed pads is much cheaper.
            vt_sb = vtp.tile([P, KT, D], bf16)
            nc.any.memzero(vt_sb[:, KT - 1, :])

            grp_idx = 0

            def evac(out_ap, psum_ap, parity):
                # rotate PSUM evacuation between ScalarE and VectorE so each
                # engine alternates the wide (512-col) and narrow (268-col)
                # bank copies across groups
                if parity % 2 == 0:
                    nc.scalar.copy(out_ap, psum_ap)
                else:
                    nc.vector.tensor_copy(out=out_ap, in_=psum_ap)

            def pass_mms(ps, src_fn, msz):
                for kc, bi, c0, c1 in kept:
                    nc.tensor.matmul(
                        ps[:msz, c0:c1],
                        src_fn(kc),
                        vt_sb[:, kc, c0:c1],
                        start=(kc == first_kc[bi]),
                        stop=(kc == last_kc[bi]),
                    )

            def load_x_a(b):
                # first half of the next batch's load; the second half goes
                # out mid-slot (load_x_b) to smooth the DMA burst against the
                # y stores
                xb = xbp.tile([P, KT, D], bf16, tag="xb")
                nc.any.memzero(xb[:, KT - 1, :])
                nc.sync.dma_start(xb[:, :3, :], x1[b, :, :3, :])
                return xb

            def load_x_b(b, xb):
                nc.sync.dma_start(xb[:, 3:KT - 1, :], x1[b, :, 3:, :])
                nc.sync.dma_start(xb[:LAST, KT - 1, :], x2[b])

            def load_x0():
                # startup: load V^T and X0 chunk-by-chunk in matmul emission
                # order so the first group starts (subtile deps) after ~0.4 MB
                # instead of after all of V^T + X0.
                xb = xbp.tile([P, KT, D], bf16, tag="xb")
                nc.any.memzero(xb[:, KT - 1, :])
                for kc in kc_order:
                    if kc == KT - 1:
                        nc.sync.dma_start(vt_sb[:LAST, kc, :], vt2)
                        nc.sync.dma_start(xb[:LAST, kc, :], x2[0])
                    else:
                        nc.sync.dma_start(vt_sb[:, kc, :], vt1[:, kc, :])
                        nc.sync.dma_start(xb[:, kc, :], x1[0, :, kc, :])
                return xb

            EV0 = 392     # balanced evac split point (copies may span banks)

            # pchunks: single-generation tiles holding PT for ALL batches,
            # pch[kc][p, b*780 + c] = PT_b[kc*128 + p, c]. Pass-2 then runs
            # over the flat 6240-row axis in 49 full-width chunks (no ragged
            # 12-row pass-2 groups). The kc=6 pad partitions are zeroed once.
            pch = [ptp.tile([P, B_LOC * D], bf16, tag=f"pt{i}", name=f"pc{i}")
                   for i in range(KT)]
            nc.any.memzero(pch[KT - 1][:])

            def emit_p1(b, mc, xb):
                nonlocal grp_idx
                m0, msz = CHUNKS[mc]
                ps = psp.tile([P, D], f32, tag="ps")
                pass_mms(ps, lambda kc: xb[:, kc, m0:m0 + msz], msz)
                dst = pch[mc][:, b * D:(b + 1) * D]
                evac(dst[:msz, :EV0], ps[:msz, :EV0], grp_idx)
                evac(dst[:msz, EV0:], ps[:msz, EV0:], grp_idx + 1)
                grp_idx += 1

            def emit_p2(j, yo_pair):
                nonlocal grp_idx
                j0, jsz = CH2[j]
                ps = psp.tile([P, D], f32, tag="ps")
                pass_mms(ps, lambda kc: pch[kc][:, j0:j0 + jsz], jsz)
                if j == NJ2 - 1:
                    yo = yop.tile([P, D], bf16, tag="yot")
                    evac(yo[:jsz, :EV0], ps[:jsz, :EV0], grp_idx)
                    evac(yo[:jsz, EV0:], ps[:jsz, EV0:], grp_idx + 1)
                    grp_idx += 1
                    nc.sync.dma_start(y2[:], yo[:jsz, :])
                    return None
                if yo_pair is None:
                    yo_pair = yop.tile([P, 4, D], bf16, tag="yo")
                t = j % 4
                evac(yo_pair[:jsz, t, :EV0], ps[:jsz, :EV0], grp_idx)
                evac(yo_pair[:jsz, t, EV0:], ps[:jsz, EV0:], grp_idx + 1)
                grp_idx += 1
                if t == 3:
                    nc.sync.dma_start(y1[j // 4], yo_pair[:])
                    return None
                return yo_pair

            # software pipeline: slot b runs pass-1 of batch b interleaved
            # with the pass-2 flat chunks that became computable after batch
            # b-1 (those reading columns < 780*b). The first pass-2 chunk of
            # a slot trails two pass-1 groups so batch b-1's last
            # evacuations have landed.
            xb_cur = load_x0()
            yo_pair = None
            q = 0
            for b in range(B_LOC):
                if b + 1 < B_LOC:
                    xb_next = load_x_a(b + 1)
                avail = (D * b) // P
                for i in range(KT):
                    emit_p1(b, i, xb_cur)
                    if i == 3 and b + 1 < B_LOC:
                        load_x_b(b + 1, xb_next)
                    if i >= 1 and q < avail:
                        yo_pair = emit_p2(q, yo_pair)
                        q += 1
                while q < avail:
                    yo_pair = emit_p2(q, yo_pair)
                    q += 1
                xb_cur = xb_next
            while q < NJ2:
                yo_pair = emit_p2(q, yo_pair)
                q += 1

    nc.compile()
    _dedupe_ldweights(nc)
    return nc


def _dedupe_ldweights(nc):
    """Drop an InstLdweights whose weights AP is identical to the previous
    one with only PE matmuls in between — the weights are already resident
    in the PE array, so the reload only burns weight-port bandwidth (the
    group bottleneck). Only sync-free loads are dropped, and matmul
    semaphore updates are untouched, so the schedule's counts are
    preserved. Runs post-compile, pre-serialization.
    """
    import concourse.mybir as mybir

    removed = 0
    for blk in nc.main_func.blocks:
        insts = blk.instructions
        last_key = None
        drop = []
        for x in insts:
            if isinstance(x, mybir.InstLdweights):
                si = x.sync_info
                clean = si is None or (len(si.on_wait) == 0
                                       and len(si.on_update) == 0)
                key = str(x.ins[0])
                if clean and key == last_key:
                    drop.append(x)
                    continue
                last_key = key
            elif not isinstance(x, mybir.InstMatmult):
                # conservatively assume anything else on the PE engine (or
                # control flow) may disturb the loaded weights
                eng = getattr(x, "engine", None)
                if eng is None or "PE" in str(eng):
                    last_key = None
        for x in drop:
            insts.remove(x)
        removed += len(drop)
    return removed


def _get_program(intervals):
    key = tuple(tuple(row) for row in intervals)
    if _CACHE.get("key") != key:
        _CACHE["nc"] = _build_program(intervals)
        _CACHE["key"] = key
    return _CACHE["nc"]


def kernel(input_state, angles, A, B, C, _trace=False):
    from concourse.bass_utils import run_bass_kernel_spmd

    X = np.asarray(input_state, dtype=np.float32)
    V = _build_V(np.asarray(angles, dtype=np.float64), np.asarray(B))
    vt = np.ascontiguousarray(V.T).astype(ml_dtypes.bfloat16)
    X_bf = X.astype(ml_dtypes.bfloat16)

    # pack: x1[b, p, kc, n] = X[b, kc*128+p, n]
    x1 = np.ascontiguousarray(
        X_bf[:, :FULL].reshape(B_TOTAL, KT - 1, P, D).transpose(0, 2, 1, 3))
    x2 = np.ascontiguousarray(X_bf[:, FULL:])
    vt1 = np.ascontiguousarray(
        vt[:FULL].reshape(KT - 1, P, D).transpose(1, 0, 2))
    vt2 = np.ascontiguousarray(vt[FULL:])

    intervals = _plan_intervals(V)
    nc = _get_program(intervals)
    in_maps = [
        {"x1": x1[c * B_LOC:(c + 1) * B_LOC],
         "x2": x2[c * B_LOC:(c + 1) * B_LOC],
         "vt1": vt1, "vt2": vt2}
        for c in range(N_CORES)
    ]
    res = run_bass_kernel_spmd(nc, in_maps, core_ids=list(range(N_CORES)),
                               trace=_trace)
    out = np.empty((B_TOTAL, D, D), np.float32)
    n_full = ((B_LOC * D) // P // 4) * 4 * P     # 48 chunks of 128 rows
    for c in range(N_CORES):
        # y1[q, p, t, n] = flat row 512*q + 128*t + p; y2 = final 96 rows;
        # flat row = b*780 + c within the core's 8 batches
        y1 = np.asarray(res.results[c]["y1"], dtype=np.float32)
        y2 = np.asarray(res.results[c]["y2"], dtype=np.float32)
        flat = np.empty((B_LOC * D, D), np.float32)
        flat[:n_full] = y1.transpose(0, 2, 1, 3).reshape(-1, D)
        flat[n_full:] = y2
        out[c * B_LOC:(c + 1) * B_LOC] = flat.reshape(B_LOC, D, D)
    if _trace:
        kernel.last_results = res
    return out
